# revision 10
# baseline (speedup 1.0000x reference)
"""Trainium2 Bass kernel for nn_BranchingLayer (gnn_message_passing).

Reference computation (shapes hardcoded from the spec):
  x:[786432,32] f32, global_features:[2048,16], parents_idxs:[524288] i32,
  W1:[48,128], b1:[128], W2:[128,128], b2:[128]
  parents = x[parents_idxs]                # [524288, 32], row i = (p, b)
  h  = leaky_relu(concat(parents, g[b]) @ W1 + b1, 0.01)
  proj = h @ W2 + b2 + repeat_interleave(parents, 4, -1)
  children[(p*4+br)*2048 + b, f] = proj[p*2048+b, br*32+f]
  out = concat([x, children], 0)           # [2883584, 32]

Design:
 * Shard the 256 parents over 8 cores (32/core); per-core x and output
   slices are contiguous.
 * fp16 matmuls (fp32 PE runs at 1/4 rate), fp32 PSUM accumulation.
   leaky(z) = 0.99*relu(z) + 0.01*z with the linear 0.01*z@W2 term folded
   into the residual matmul weights (host-precomputed in f64).
 * K=128 everywhere: K<128 matmuls measure ~3x slower per free-column on
   HW, so two parents share one [128, 2048] xt tile (A rows 0-48, B rows
   49-97; rows 98-127 zeroed once) and every matmul streams the full 128
   partitions.  Parent selection happens in the stationary operand: wA
   has the weights in rows 0-48 with rows 49+ zero, wB in rows 49-97 --
   the other parent's data multiplies zeros.  Stationary swaps are free
   (double-buffered weight load).
 * Feature-major compute: per parent/quarter, psum1[128f,512] =
   w^T.xt (bias via ones row), h1 = relu(psum1) (ACT, fp16),
   psum2[128j,512] = W2^T.h1 + er^T.xt (residual + lin + biases);
   DVE 32x32 block-transpose psum2 -> bt (f32).  The output DMA casts
   f32->fp16 in flight (SWDGE CME cast): the 2e-2 rel-err budget easily
   covers fp16 children, and halving the output bytes halves the
   dominant HBM write traffic; the host upcasts when assembling.
   mm1+relu of parent p+1 are emitted between parent p's mm2 groups so
   PE never waits on ACT.
 * Batch columns are host-permuted: position 32c+d holds row 64d+c.
   After the 32x32 block transpose, partition 32*br+d of bt holds batch
   rows {64d+c} of branch br as one contiguous 8KB DRAM chunk, and the
   chunks are partition-ordered -> the whole parent is ONE 128-partition
   1MB contiguous output DMA (all 16 SDMA engines engaged) on the
   otherwise-idle GPSIMD (SWDGE) ring.
"""

import numpy as np

BATCH = 2048
NPAR = 256
NF = 32
NG = 16
NBR = 4
OFF = 262144
NCORES = 8
PPC = NPAR // NCORES          # parents per core
QW = 512                      # matmul free-dim (quarter of batch)
NQ = BATCH // QW
XROWS = 49                    # 0-31 x, 32-47 g, 48 ones

_CACHE = {}


def _build_nc(ppc=PPC, reps=1):
    import concourse.bacc as bacc
    import concourse.bass as bass
    import concourse.mybir as mybir
    import concourse.tile as tile
    from contextlib import ExitStack, nullcontext

    bf = mybir.dt.float16
    f32 = mybir.dt.float32
    nc = bacc.Bacc("TRN2", target_bir_lowering=False, debug=False)

    npair = ppc // 2
    xt_d = nc.dram_tensor("xt", [npair, 2 * XROWS, BATCH], bf, kind="ExternalInput")
    w1e_d = nc.dram_tensor("w1e", [128, 128], bf, kind="ExternalInput")
    w1o_d = nc.dram_tensor("w1o", [128, 128], bf, kind="ExternalInput")
    ere_d = nc.dram_tensor("ere", [128, 128], bf, kind="ExternalInput")
    ero_d = nc.dram_tensor("ero", [128, 128], bf, kind="ExternalInput")
    w2_d = nc.dram_tensor("w2", [128, 128], bf, kind="ExternalInput")
    out_d = nc.dram_tensor("out", [ppc, 128, BATCH], bf, kind="ExternalOutput")

    with tile.TileContext(nc) as tc, ExitStack() as ctx:
        wpool = ctx.enter_context(tc.tile_pool(name="w", bufs=1))
        hpool = ctx.enter_context(tc.tile_pool(name="h", bufs=8))
        btpool = ctx.enter_context(tc.tile_pool(name="bt", bufs=6))
        p1pool = ctx.enter_context(
            tc.tile_pool(name="p1", bufs=4, space=bass.MemorySpace.PSUM)
        )
        p2pool = ctx.enter_context(
            tc.tile_pool(name="p2", bufs=4, space=bass.MemorySpace.PSUM)
        )

        w_t = {}
        for name, dram in (
            ("w1e", w1e_d), ("w1o", w1o_d), ("ere", ere_d), ("ero", ero_d),
            ("w2", w2_d),
        ):
            t = wpool.tile([128, 128], bf, tag=name)
            nc.sync.dma_start(t[:], dram[:])
            w_t[name] = t

        NXT = 4
        xt_static = []
        for i in range(NXT):
            t = wpool.tile([128, BATCH], bf, tag=f"xts{i}")
            nc.vector.memset(t[:], 0.0)
            xt_static.append(t)

        rep_ctx = tc.For_i(0, reps, 1) if reps > 1 else nullcontext()
        with rep_ctx:
            h1_t = {}

            def load_pair(j):
                if j >= npair:
                    return
                nc.sync.dma_start(xt_static[j % NXT][: 2 * XROWS, :], xt_d[j])

            def xt_tile(p):
                return xt_static[(p // 2) % NXT]

            def mm1(p, q):
                s = slice(q * QW, (q + 1) * QW)
                w1 = w_t["w1e"] if p % 2 == 0 else w_t["w1o"]
                ps1 = p1pool.tile([128, QW], f32, tag="ps1")
                nc.tensor.matmul(ps1[:], w1[:], xt_tile(p)[:, s], start=True, stop=True)
                h1 = hpool.tile([128, QW], bf, tag="h1")
                nc.scalar.activation(h1[:], ps1[:], mybir.ActivationFunctionType.Relu)
                h1_t[(p, q)] = h1

            def mm2(p, q, bt_t):
                s = slice(q * QW, (q + 1) * QW)
                er = w_t["ere"] if p % 2 == 0 else w_t["ero"]
                ps2 = p2pool.tile([128, QW], f32, tag="ps2")
                nc.tensor.matmul(
                    ps2[:], w_t["w2"][:], h1_t.pop((p, q))[:], start=True, stop=False
                )
                nc.tensor.matmul(ps2[:], er[:], xt_tile(p)[:, s], start=False, stop=True)
                nc.vector.transpose(bt_t[:, s], ps2[:])

            load_pair(0)
            load_pair(1)
            for q in range(NQ):
                mm1(0, q)
            for p in range(ppc):
                if p % 2 == 0:
                    load_pair(p // 2 + 2)
                bt_t = btpool.tile([128, BATCH], f32, tag="bt")
                for half in range(2):
                    for q in (0, 1) if half == 0 else (2, 3):
                        mm2(p, q, bt_t)
                    if p + 1 < ppc:
                        for q in (0, 1) if half == 0 else (2, 3):
                            mm1(p + 1, q)
                nc.gpsimd.dma_start(out_d[p], bt_t[:])
    nc.compile()
    return nc


def _get_nc():
    if "nc" not in _CACHE:
        _CACHE["nc"] = _build_nc()
    return _CACHE["nc"]


def _perm_cols(a):
    """Permute the trailing batch axis: position 32c+d <- row 64d+c."""
    shp = a.shape[:-1]
    return np.ascontiguousarray(
        a.reshape(*shp, 32, 64).swapaxes(-1, -2).reshape(*shp, BATCH)
    )


def _pack_inputs(x, global_features, parents_idxs, W1, b1, W2, b2, ppc=PPC):
    """Build the per-core input maps (host-side sharding + layout)."""
    bf16 = np.float16
    x = np.asarray(x, np.float32)
    g = np.asarray(global_features, np.float32)
    idx = np.asarray(parents_idxs)
    W1 = np.asarray(W1, np.float32)
    b1 = np.asarray(b1, np.float32)
    W2 = np.asarray(W2, np.float32)
    b2 = np.asarray(b2, np.float32)

    n_rows = NPAR * BATCH
    exp = np.arange(n_rows, dtype=np.int64)
    if np.array_equal(idx, exp + OFF):
        parents = x[OFF : OFF + n_rows]
    else:
        parents = x[idx]  # general gather
    gi = idx.astype(np.int64) % BATCH
    if not np.array_equal(gi, np.tile(np.arange(BATCH, dtype=np.int64), NPAR)):
        return None

    # Feature-major per-parent x with permuted batch columns
    xf = parents.reshape(NPAR, BATCH, NF).transpose(0, 2, 1)  # [P, 32, B]
    xf = _perm_cols(xf)
    g_hi = _perm_cols(np.ascontiguousarray(g.T)).astype(bf16)  # [16, B]

    xt = np.empty((NPAR, XROWS, BATCH), bf16)
    xt[:, :32] = xf.astype(bf16)
    xt[:, 32:48] = g_hi[None]
    xt[:, 48] = np.float32(1.0)
    # pair layout: [npair_total, 98, B] -- parent 2j rows 0-48, 2j+1 rows 49-97
    xtp = xt.reshape(NPAR // 2, 2 * XROWS, BATCH)

    W1f = W1.astype(np.float64)
    W2f = W2.astype(np.float64)
    lin = 0.01 * (W1f @ W2f)  # [48, 128]
    w1 = np.zeros((XROWS, 128), np.float32)
    w1[:48] = W1
    w1[48] = b1
    er = np.zeros((XROWS, 128), np.float64)
    jj = np.arange(128)
    er[jj // 4, jj] = 1.0
    er[:48] += lin
    er[48] = b2.astype(np.float64) + 0.01 * (b1.astype(np.float64) @ W2f)

    def pad128(m, row0):
        out = np.zeros((128, 128), np.float32)
        out[row0 : row0 + XROWS] = m
        return out.astype(bf16)

    w1e = pad128(w1, 0)
    w1o = pad128(w1, XROWS)
    ere = pad128(er, 0)
    ero = pad128(er, XROWS)
    w2 = (0.99 * W2f).astype(bf16)

    ncores = NPAR // ppc
    npair = ppc // 2
    in_maps = []
    for c in range(ncores):
        in_maps.append(
            {
                "xt": xtp[c * npair : (c + 1) * npair],
                "w1e": w1e,
                "w1o": w1o,
                "ere": ere,
                "ero": ero,
                "w2": w2,
            }
        )
    return in_maps


def _numpy_fallback(x, global_features, parents_idxs, W1, b1, W2, b2):
    x = np.asarray(x, np.float32)
    g = np.asarray(global_features, np.float32)
    idx = np.asarray(parents_idxs).astype(np.int64)
    pf = x[idx]
    pg = g[idx % BATCH]
    h = np.concatenate([pf, pg], axis=-1) @ np.asarray(W1, np.float32) + b1
    h = np.where(h > 0, h, 0.01 * h).astype(np.float32)
    proj = h @ np.asarray(W2, np.float32) + b2
    proj = proj + np.repeat(pf, NBR, axis=-1)
    m = proj.reshape(NPAR, BATCH, NF * NBR)
    m = np.swapaxes(m, 1, 2)
    m = m.reshape(NPAR * NBR, NF, BATCH)
    m = np.swapaxes(m, 1, 2)
    children = m.reshape(NPAR * NBR * BATCH, NF)
    return np.concatenate([x, children], axis=0).astype(np.float32)


def kernel(x, global_features, parents_idxs, W1, b1, W2, b2):
    in_maps = _pack_inputs(x, global_features, parents_idxs, W1, b1, W2, b2)
    if in_maps is None:
        return _numpy_fallback(x, global_features, parents_idxs, W1, b1, W2, b2)

    from concourse.bass_utils import run_bass_kernel_spmd

    nc = _get_nc()
    res = run_bass_kernel_spmd(nc, in_maps, core_ids=list(range(NCORES)))
    _CACHE["last_result"] = res

    x = np.asarray(x, np.float32)
    out = np.empty((x.shape[0] + NPAR * NBR * BATCH, NF), np.float32)
    out[: x.shape[0]] = x
    base = x.shape[0]
    per = PPC * NBR * BATCH
    for c in range(NCORES):
        out[base + c * per : base + (c + 1) * per] = res.results[c]["out"].reshape(
            per, NF
        )
    return out


# revision 11
# speedup vs baseline: 1.2501x; 1.2501x over previous
"""Trainium2 Bass kernel for nn_BranchingLayer (gnn_message_passing).

Reference computation (shapes hardcoded from the spec):
  x:[786432,32] f32, global_features:[2048,16], parents_idxs:[524288] i32,
  W1:[48,128], b1:[128], W2:[128,128], b2:[128]
  parents = x[parents_idxs]                # [524288, 32], row i = (p, b)
  h  = leaky_relu(concat(parents, g[b]) @ W1 + b1, 0.01)
  proj = h @ W2 + b2 + repeat_interleave(parents, 4, -1)
  children[(p*4+br)*2048 + b, f] = proj[p*2048+b, br*32+f]
  out = concat([x, children], 0)           # [2883584, 32]

Design:
 * Shard the 256 parents over 8 cores (32/core); per-core x and output
   slices are contiguous.
 * fp16 matmuls (fp32 PE runs at 1/4 rate), fp32 PSUM accumulation.
   leaky(z) = 0.99*relu(z) + 0.01*z with the linear 0.01*z@W2 term folded
   into the residual matmul weights (host-precomputed in f64).
 * K=128 everywhere: K<128 matmuls measure ~3x slower per free-column on
   HW, so two parents share one [128, 2048] xt tile (A rows 0-48, B rows
   49-97; rows 98-127 zeroed once) and every matmul streams the full 128
   partitions.  Parent selection happens in the stationary operand: wA
   has the weights in rows 0-48 with rows 49+ zero, wB in rows 49-97 --
   the other parent's data multiplies zeros.  Stationary swaps are free
   (double-buffered weight load).
 * Feature-major compute: per parent/quarter, psum1[128f,512] =
   w^T.xt (bias via ones row), h1 = relu(psum1) (ACT, fp16),
   psum2[128j,512] = W2^T.h1 + er^T.xt (residual + lin + biases);
   DVE 32x32 block-transpose psum2 -> bt (f32).  The output DMA casts
   f32->fp16 in flight (SWDGE CME cast): the 2e-2 rel-err budget easily
   covers fp16 children, and halving the output bytes halves the
   dominant HBM write traffic; the host upcasts when assembling.
   mm1+relu of parent p+1 are emitted between parent p's mm2 groups so
   PE never waits on ACT.
 * Batch columns are host-permuted: position 32c+d holds row 64d+c.
   After the 32x32 block transpose, partition 32*br+d of bt holds batch
   rows {64d+c} of branch br as one contiguous 8KB DRAM chunk, and the
   chunks are partition-ordered -> the whole parent is ONE 128-partition
   1MB contiguous output DMA (all 16 SDMA engines engaged) on the
   otherwise-idle GPSIMD (SWDGE) ring.
"""

import numpy as np

BATCH = 2048
NPAR = 256
NF = 32
NG = 16
NBR = 4
OFF = 262144
NCORES = 8
PPC = NPAR // NCORES          # parents per core
QW = 512                      # matmul free-dim (quarter of batch)
NQ = BATCH // QW
XROWS = 49                    # 0-31 x, 32-47 g, 48 ones

_CACHE = {}


def _build_nc(ppc=PPC, reps=1):
    import concourse.bacc as bacc
    import concourse.bass as bass
    import concourse.mybir as mybir
    import concourse.tile as tile
    from contextlib import ExitStack, nullcontext

    bf = mybir.dt.float16
    f32 = mybir.dt.float32
    nc = bacc.Bacc("TRN2", target_bir_lowering=False, debug=False)

    npair = ppc // 2
    xt_d = nc.dram_tensor("xt", [npair, 2 * XROWS, BATCH], bf, kind="ExternalInput")
    w1e_d = nc.dram_tensor("w1e", [128, 128], bf, kind="ExternalInput")
    w1o_d = nc.dram_tensor("w1o", [128, 128], bf, kind="ExternalInput")
    ere_d = nc.dram_tensor("ere", [128, 128], bf, kind="ExternalInput")
    ero_d = nc.dram_tensor("ero", [128, 128], bf, kind="ExternalInput")
    w2_d = nc.dram_tensor("w2", [128, 128], bf, kind="ExternalInput")
    out_d = nc.dram_tensor("out", [ppc, 128, BATCH], bf, kind="ExternalOutput")

    with tile.TileContext(nc) as tc, ExitStack() as ctx:
        wpool = ctx.enter_context(tc.tile_pool(name="w", bufs=1))
        hpool = ctx.enter_context(tc.tile_pool(name="h", bufs=8))
        btpool = ctx.enter_context(tc.tile_pool(name="bt", bufs=4))
        p1pool = ctx.enter_context(
            tc.tile_pool(name="p1", bufs=4, space=bass.MemorySpace.PSUM)
        )
        p2pool = ctx.enter_context(
            tc.tile_pool(name="p2", bufs=3, space=bass.MemorySpace.PSUM)
        )

        w_t = {}
        for name, dram in (
            ("w1e", w1e_d), ("w1o", w1o_d), ("ere", ere_d), ("ero", ero_d),
            ("w2", w2_d),
        ):
            t = wpool.tile([128, 128], bf, tag=name)
            nc.sync.dma_start(t[:], dram[:])
            w_t[name] = t

        NXT = 3
        xt_static = []
        for i in range(NXT):
            t = wpool.tile([128, BATCH], bf, tag=f"xts{i}")
            nc.vector.memset(t[:], 0.0)
            xt_static.append(t)

        rep_ctx = tc.For_i(0, reps, 1) if reps > 1 else nullcontext()
        with rep_ctx:
            h1_t = {}

            def load_pair(j):
                if j >= npair:
                    return
                nc.sync.dma_start(xt_static[j % NXT][: 2 * XROWS, :], xt_d[j])

            def xt_tile(p):
                return xt_static[(p // 2) % NXT]

            def mm1(p, q):
                s = slice(q * QW, (q + 1) * QW)
                w1 = w_t["w1e"] if p % 2 == 0 else w_t["w1o"]
                ps1 = p1pool.tile([128, QW], f32, tag="ps1")
                nc.tensor.matmul(ps1[:], w1[:], xt_tile(p)[:, s], start=True, stop=True)
                h1 = hpool.tile([128, QW], bf, tag="h1")
                nc.scalar.activation(h1[:], ps1[:], mybir.ActivationFunctionType.Relu)
                h1_t[(p, q)] = h1

            def mm2(p, q, bt_t):
                s = slice(q * QW, (q + 1) * QW)
                er = w_t["ere"] if p % 2 == 0 else w_t["ero"]
                ps2 = p2pool.tile([128, QW], f32, tag="ps2")
                nc.tensor.matmul(
                    ps2[:], w_t["w2"][:], h1_t.pop((p, q))[:], start=True, stop=False
                )
                nc.tensor.matmul(ps2[:], er[:], xt_tile(p)[:, s], start=False, stop=True)
                nc.vector.transpose(bt_t[:, s], ps2[:])

            load_pair(0)
            load_pair(1)
            for q in range(NQ):
                mm1(0, q)
            for p in range(ppc):
                if p % 2 == 0:
                    load_pair(p // 2 + 2)
                bt_t = btpool.tile([128, BATCH], f32, tag="bt")
                for half in range(2):
                    for q in (0, 1) if half == 0 else (2, 3):
                        mm2(p, q, bt_t)
                    if p + 1 < ppc:
                        for q in (0, 1) if half == 0 else (2, 3):
                            mm1(p + 1, q)
                nc.gpsimd.dma_start(out_d[p], bt_t[:])
    nc.compile()
    return nc


def _get_nc():
    if "nc" not in _CACHE:
        _CACHE["nc"] = _build_nc()
    return _CACHE["nc"]


def _perm_cols(a):
    """Permute the trailing batch axis: position 32c+d <- row 64d+c."""
    shp = a.shape[:-1]
    return np.ascontiguousarray(
        a.reshape(*shp, 32, 64).swapaxes(-1, -2).reshape(*shp, BATCH)
    )


def _pack_inputs(x, global_features, parents_idxs, W1, b1, W2, b2, ppc=PPC):
    """Build the per-core input maps (host-side sharding + layout)."""
    bf16 = np.float16
    x = np.asarray(x, np.float32)
    g = np.asarray(global_features, np.float32)
    idx = np.asarray(parents_idxs)
    W1 = np.asarray(W1, np.float32)
    b1 = np.asarray(b1, np.float32)
    W2 = np.asarray(W2, np.float32)
    b2 = np.asarray(b2, np.float32)

    n_rows = NPAR * BATCH
    exp = np.arange(n_rows, dtype=np.int64)
    if np.array_equal(idx, exp + OFF):
        parents = x[OFF : OFF + n_rows]
    else:
        parents = x[idx]  # general gather
    gi = idx.astype(np.int64) % BATCH
    if not np.array_equal(gi, np.tile(np.arange(BATCH, dtype=np.int64), NPAR)):
        return None

    # Feature-major per-parent x with permuted batch columns
    xf = parents.reshape(NPAR, BATCH, NF).transpose(0, 2, 1)  # [P, 32, B]
    xf = _perm_cols(xf)
    g_hi = _perm_cols(np.ascontiguousarray(g.T)).astype(bf16)  # [16, B]

    xt = np.empty((NPAR, XROWS, BATCH), bf16)
    xt[:, :32] = xf.astype(bf16)
    xt[:, 32:48] = g_hi[None]
    xt[:, 48] = np.float32(1.0)
    # pair layout: [npair_total, 98, B] -- parent 2j rows 0-48, 2j+1 rows 49-97
    xtp = xt.reshape(NPAR // 2, 2 * XROWS, BATCH)

    W1f = W1.astype(np.float64)
    W2f = W2.astype(np.float64)
    lin = 0.01 * (W1f @ W2f)  # [48, 128]
    w1 = np.zeros((XROWS, 128), np.float32)
    w1[:48] = W1
    w1[48] = b1
    er = np.zeros((XROWS, 128), np.float64)
    jj = np.arange(128)
    er[jj // 4, jj] = 1.0
    er[:48] += lin
    er[48] = b2.astype(np.float64) + 0.01 * (b1.astype(np.float64) @ W2f)

    def pad128(m, row0):
        out = np.zeros((128, 128), np.float32)
        out[row0 : row0 + XROWS] = m
        return out.astype(bf16)

    w1e = pad128(w1, 0)
    w1o = pad128(w1, XROWS)
    ere = pad128(er, 0)
    ero = pad128(er, XROWS)
    w2 = (0.99 * W2f).astype(bf16)

    ncores = NPAR // ppc
    npair = ppc // 2
    in_maps = []
    for c in range(ncores):
        in_maps.append(
            {
                "xt": xtp[c * npair : (c + 1) * npair],
                "w1e": w1e,
                "w1o": w1o,
                "ere": ere,
                "ero": ero,
                "w2": w2,
            }
        )
    return in_maps


def _numpy_fallback(x, global_features, parents_idxs, W1, b1, W2, b2):
    x = np.asarray(x, np.float32)
    g = np.asarray(global_features, np.float32)
    idx = np.asarray(parents_idxs).astype(np.int64)
    pf = x[idx]
    pg = g[idx % BATCH]
    h = np.concatenate([pf, pg], axis=-1) @ np.asarray(W1, np.float32) + b1
    h = np.where(h > 0, h, 0.01 * h).astype(np.float32)
    proj = h @ np.asarray(W2, np.float32) + b2
    proj = proj + np.repeat(pf, NBR, axis=-1)
    m = proj.reshape(NPAR, BATCH, NF * NBR)
    m = np.swapaxes(m, 1, 2)
    m = m.reshape(NPAR * NBR, NF, BATCH)
    m = np.swapaxes(m, 1, 2)
    children = m.reshape(NPAR * NBR * BATCH, NF)
    return np.concatenate([x, children], axis=0).astype(np.float32)


def kernel(x, global_features, parents_idxs, W1, b1, W2, b2):
    in_maps = _pack_inputs(x, global_features, parents_idxs, W1, b1, W2, b2)
    if in_maps is None:
        return _numpy_fallback(x, global_features, parents_idxs, W1, b1, W2, b2)

    from concourse.bass_utils import run_bass_kernel_spmd

    nc = _get_nc()
    res = run_bass_kernel_spmd(nc, in_maps, core_ids=list(range(NCORES)))
    _CACHE["last_result"] = res

    x = np.asarray(x, np.float32)
    out = np.empty((x.shape[0] + NPAR * NBR * BATCH, NF), np.float32)
    out[: x.shape[0]] = x
    base = x.shape[0]
    per = PPC * NBR * BATCH
    for c in range(NCORES):
        out[base + c * per : base + (c + 1) * per] = res.results[c]["out"].reshape(
            per, NF
        )
    return out
